# revision 3
# baseline (speedup 1.0000x reference)
"""GAT (2-layer, 8-head) message-passing kernel for Trainium2, 8 NeuronCores.

Strategy
--------
Nodes are partitioned into 8 contiguous ranges (one per core).  Edges
(incl. self-loops) are routed on the host to the core that owns their
destination node, sorted by destination, and padded into 128-edge
"subtiles" grouped under 128-node "dst blocks" so the segment softmax and
scatter-add become dense block-local ops:

  * per-edge messages g[src] are fetched with one big indirect DMA per
    dst block (K*128 row gathers of 512B each),
  * exp(leaky_relu(a_src+a_dst)) edge weights are computed on-chip,
  * the weighted scatter-add over a block's edges is a PSUM-accumulated
    matmul with an on-chip-built 0/1 selection matrix S[e, dst_local],
    which also accumulates the softmax denominators as 8 extra columns.

Three SPMD launches (no collectives; the host re-shards between phases):
  phase 1: x @ W_emb -> h0; g0 = h0 @ W0; att scalars     (per-node, tiny)
  phase 2: GAT layer 0 over g0 table -> g1 table + att scalars
  phase 3: GAT layer 1 over g1 table -> causal_effect, propensity heads
"""

import math
import os
import sys

import numpy as np

for _p in ("/opt/trn_rl_repo",):
    if _p not in sys.path and os.path.isdir(_p):
        sys.path.append(_p)

import concourse.bass as bass
import concourse.mybir as mybir
import concourse.tile as tile
from concourse import bacc
from concourse.bass import ds, ts

f32 = mybir.dt.float32
i32 = mybir.dt.int32

# ---------------------------------------------------------------- problem dims
N = 100000       # nodes
E = 1600000      # edges (before self loops)
F = 128          # raw features
D = 16           # per-head dim
H = 8            # heads
HD = H * D       # 128
NEG_SLOPE = 0.2
EPS = 1e-16

NCORES = 8
P = 128

# node padding / sharding (hardcoded for the real problem; overridable for dev)
class _Cfg:
    def __init__(self, n=N, ncores=NCORES):
        self.n = n
        self.ncores = ncores
        self.npc = int(math.ceil(n / ncores / P)) * P   # nodes per core
        self.nb = self.npc // P                          # dst blocks per core
        self.npad = self.npc * ncores                    # padded total nodes


CFG = _Cfg()

# filled by kernel() for test.py to inspect
LAST_EXEC_TIMES = {}
LAST_TRACES = {}
DEBUG_STASH = {}


# =====================================================================
# bass program builders
# =====================================================================

def _mk_nc():
    return bacc.Bacc("TRN2", target_bir_lowering=False, debug=False)


def _build_phase1(cfg):
    """Per-core: h0 = x @ W_emb + b_emb ; g0 = h0 @ W0 ; asd0 = h0 @ Was0cat."""
    nc = _mk_nc()
    NPC, NB = cfg.npc, cfg.nb

    xT = nc.dram_tensor("xT", [F, NPC], f32, kind="ExternalInput")
    Wemb = nc.dram_tensor("Wemb", [F, D], f32, kind="ExternalInput")
    bemb = nc.dram_tensor("bemb", [D, 1], f32, kind="ExternalInput")
    W0 = nc.dram_tensor("W0", [D, HD], f32, kind="ExternalInput")
    Was0 = nc.dram_tensor("Was0", [D, 2 * H], f32, kind="ExternalInput")
    g_out = nc.dram_tensor("g_out", [NPC, HD], f32, kind="ExternalOutput")
    asd_out = nc.dram_tensor("asd_out", [NPC, 2 * H], f32, kind="ExternalOutput")

    with tile.TileContext(nc) as tc:
        with (
            tc.tile_pool(name="const", bufs=1) as cpool,
            tc.tile_pool(name="work", bufs=3) as wpool,
            tc.tile_pool(name="ps", bufs=2, space="PSUM") as ppool,
        ):
            Wemb_sb = cpool.tile([F, D], f32)
            nc.sync.dma_start(out=Wemb_sb[:], in_=Wemb[:, :])
            bemb_sb = cpool.tile([D, 1], f32)
            nc.sync.dma_start(out=bemb_sb[:], in_=bemb[:, :])
            W0_sb = cpool.tile([D, HD], f32)
            nc.sync.dma_start(out=W0_sb[:], in_=W0[:, :])
            Was0_sb = cpool.tile([D, 2 * H], f32)
            nc.sync.dma_start(out=Was0_sb[:], in_=Was0[:, :])

            for b in range(NB):
                xT_blk = wpool.tile([F, P], f32, tag="xT")
                nc.sync.dma_start(out=xT_blk[:], in_=xT[:, b * P:(b + 1) * P])

                h0T_ps = ppool.tile([D, P], f32, tag="h0T")
                nc.tensor.matmul(out=h0T_ps[:], lhsT=Wemb_sb[:], rhs=xT_blk[:],
                                 start=True, stop=True)
                h0T_sb = wpool.tile([D, P], f32, tag="h0T_sb")
                nc.scalar.activation(out=h0T_sb[:], in_=h0T_ps[:],
                                     func=mybir.ActivationFunctionType.Identity,
                                     bias=bemb_sb[:, 0:1], scale=1.0)

                g0_ps = ppool.tile([P, HD], f32, tag="g0")
                nc.tensor.matmul(out=g0_ps[:], lhsT=h0T_sb[:], rhs=W0_sb[:],
                                 start=True, stop=True)
                asd_ps = ppool.tile([P, 2 * H], f32, tag="asd")
                nc.tensor.matmul(out=asd_ps[:], lhsT=h0T_sb[:], rhs=Was0_sb[:],
                                 start=True, stop=True)

                g_st = wpool.tile([P, HD], f32, tag="g_st")
                nc.vector.tensor_copy(out=g_st[:], in_=g0_ps[:])
                asd_st = wpool.tile([P, 2 * H], f32, tag="asd_st")
                nc.scalar.activation(out=asd_st[:], in_=asd_ps[:],
                                     func=mybir.ActivationFunctionType.Identity,
                                     bias=0.0, scale=1.0)

                nc.sync.dma_start(out=g_out[b * P:(b + 1) * P, :], in_=g_st[:])
                nc.sync.dma_start(out=asd_out[b * P:(b + 1) * P, :], in_=asd_st[:])
    nc.compile()
    return nc


def _build_gat(cfg, K, phase):
    """GAT aggregation layer.  phase=2 emits (g1, asd1); phase=3 emits heads."""
    assert phase in (2, 3)
    nc = _mk_nc()
    NPC, NB, NPAD = cfg.npc, cfg.nb, cfg.npad
    C = HD + H                     # rhs cols per subtile: 128 msg + 8 weight
    KC = K * C

    gtable = nc.dram_tensor("gtable", [NPAD, HD], f32, kind="ExternalInput")
    idx = nc.dram_tensor("idx", [NPC, K], i32, kind="ExternalInput")
    dstl = nc.dram_tensor("dstl", [NPC, K], f32, kind="ExternalInput")
    stream = nc.dram_tensor("stream", [NPC, K * 2 * H], f32, kind="ExternalInput")
    iot = nc.dram_tensor("iota", [P, P], f32, kind="ExternalInput")
    ident = nc.dram_tensor("ident", [P, P], f32, kind="ExternalInput")
    bcol = nc.dram_tensor("bcol", [HD, 1], f32, kind="ExternalInput")
    if phase == 2:
        W1 = nc.dram_tensor("W1", [HD, HD], f32, kind="ExternalInput")
        Was1 = nc.dram_tensor("Was1", [HD, 2 * H], f32, kind="ExternalInput")
        g_out = nc.dram_tensor("g_out", [NPC, HD], f32, kind="ExternalOutput")
        asd_out = nc.dram_tensor("asd_out", [NPC, 2 * H], f32, kind="ExternalOutput")
    else:
        dWy = nc.dram_tensor("dWy", [HD, 1], f32, kind="ExternalInput")
        Wc1 = nc.dram_tensor("Wc1", [HD, D], f32, kind="ExternalInput")
        dby = nc.dram_tensor("dby", [1, 1], f32, kind="ExternalInput")
        bc1 = nc.dram_tensor("bc1", [D, 1], f32, kind="ExternalInput")
        bc2 = nc.dram_tensor("bc2", [1, 1], f32, kind="ExternalInput")
        Wc2 = nc.dram_tensor("Wc2", [D, 1], f32, kind="ExternalInput")
        ce_out = nc.dram_tensor("ce_out", [1, NPC], f32, kind="ExternalOutput")
        prop_out = nc.dram_tensor("prop_out", [1, NPC], f32, kind="ExternalOutput")

    AF = mybir.ActivationFunctionType
    with tile.TileContext(nc) as tc:
        with (
            tc.tile_pool(name="const", bufs=1) as cpool,
            tc.tile_pool(name="work", bufs=2) as wpool,
            tc.tile_pool(name="ps2", bufs=2, space="PSUM") as ppool2,
            tc.tile_pool(name="ps1", bufs=1, space="PSUM") as ppool1,
        ):
            iota_sb = cpool.tile([P, P], f32)
            nc.sync.dma_start(out=iota_sb[:], in_=iot[:, :])
            ident_sb = cpool.tile([P, P], f32)
            nc.sync.dma_start(out=ident_sb[:], in_=ident[:, :])
            bcol_sb = cpool.tile([HD, 1], f32)
            nc.sync.dma_start(out=bcol_sb[:], in_=bcol[:, :])
            eps_sb = cpool.tile([P, 1], f32)
            nc.vector.memset(eps_sb[:], EPS)
            if phase == 2:
                W1_sb = cpool.tile([HD, HD], f32)
                nc.sync.dma_start(out=W1_sb[:], in_=W1[:, :])
                Was1_sb = cpool.tile([HD, 2 * H], f32)
                nc.sync.dma_start(out=Was1_sb[:], in_=Was1[:, :])
            else:
                dWy_sb = cpool.tile([HD, 1], f32)
                nc.sync.dma_start(out=dWy_sb[:], in_=dWy[:, :])
                Wc1_sb = cpool.tile([HD, D], f32)
                nc.sync.dma_start(out=Wc1_sb[:], in_=Wc1[:, :])
                dby_sb = cpool.tile([1, 1], f32)
                nc.sync.dma_start(out=dby_sb[:], in_=dby[:, :])
                bc1_sb = cpool.tile([D, 1], f32)
                nc.sync.dma_start(out=bc1_sb[:], in_=bc1[:, :])
                bc2_sb = cpool.tile([1, 1], f32)
                nc.sync.dma_start(out=bc2_sb[:], in_=bc2[:, :])
                Wc2_sb = cpool.tile([D, 1], f32)
                nc.sync.dma_start(out=Wc2_sb[:], in_=Wc2[:, :])
                ce_st = cpool.tile([1, NPC], f32)
                prop_st = cpool.tile([1, NPC], f32)

            def body(b):
                # ---- per-block loads
                idx_t = wpool.tile([P, K], i32, tag="idx")
                nc.sync.dma_start(out=idx_t[:], in_=idx[ts(b, P), :])
                dstl_t = wpool.tile([P, K], f32, tag="dstl")
                nc.sync.dma_start(out=dstl_t[:], in_=dstl[ts(b, P), :])
                str_t = wpool.tile([P, K * 2 * H], f32, tag="stream")
                nc.sync.dma_start(out=str_t[:], in_=stream[ts(b, P), :])

                # ---- gather messages: G[p, k*128:(k+1)*128] = gtable[idx[p,k], :]
                # HW indirect DMA consumes ONE offset per partition row, so
                # issue one gather per 128-edge subtile (verified semantics).
                G_t = wpool.tile([P, K * HD], f32, tag="G")
                for k in range(K):
                    nc.gpsimd.indirect_dma_start(
                        out=G_t[:, k * HD:(k + 1) * HD],
                        out_offset=None,
                        in_=gtable[:, :],
                        in_offset=bass.IndirectOffsetOnAxis(
                            ap=idx_t[:, k:k + 1], axis=0),
                    )

                # ---- edge weights w = exp(leaky_relu(a_src + a_dst))
                strR = str_t[:].rearrange("p (k c) -> p k c", c=2 * H)
                logits_t = wpool.tile([P, K * H], f32, tag="logits")
                logitsR = logits_t[:].rearrange("p (k h) -> p k h", h=H)
                nc.vector.tensor_tensor(out=logitsR, in0=strR[:, :, 0:H],
                                        in1=strR[:, :, H:2 * H],
                                        op=mybir.AluOpType.add)
                lr_t = wpool.tile([P, K * H], f32, tag="lr")
                nc.vector.scalar_tensor_tensor(
                    out=lr_t[:], in0=logits_t[:], scalar=NEG_SLOPE, in1=logits_t[:],
                    op0=mybir.AluOpType.mult, op1=mybir.AluOpType.max)

                rhs_t = wpool.tile([P, KC], f32, tag="rhs")
                R = rhs_t[:].rearrange("p (k c) -> p k c", c=C)
                nc.scalar.activation(out=R[:, :, HD:C],
                                     in_=lr_t[:].rearrange("p (k h) -> p k h", h=H),
                                     func=AF.Exp, bias=0.0, scale=1.0)

                # ---- weighted messages rhs[:, :128] = G * w  (w bcast over d)
                nc.vector.tensor_tensor(
                    out=R[:, :, 0:HD].rearrange("p k (h d) -> p k h d", h=H),
                    in0=G_t[:].rearrange("p (k h d) -> p k h d", k=K, h=H),
                    in1=R[:, :, HD:C].unsqueeze(3).to_broadcast([P, K, H, D]),
                    op=mybir.AluOpType.mult)

                # ---- selection matrix S[e, k, i] = (dstl[e,k] == i)
                S_t = wpool.tile([P, K * P], f32, tag="S")
                nc.vector.tensor_tensor(
                    out=S_t[:].rearrange("p (k i) -> p k i", i=P),
                    in0=dstl_t[:].unsqueeze(2).to_broadcast([P, K, P]),
                    in1=iota_sb[:].unsqueeze(1).to_broadcast([P, K, P]),
                    op=mybir.AluOpType.is_equal)

                # ---- scatter-add into psum: out[i, :] += sum_e S[e,i] rhs[e, :]
                acc_ps = ppool2.tile([P, C], f32, tag="acc")
                for k in range(K):
                    nc.tensor.matmul(out=acc_ps[:],
                                     lhsT=S_t[:, k * P:(k + 1) * P],
                                     rhs=rhs_t[:, k * C:(k + 1) * C],
                                     start=(k == 0), stop=(k == K - 1))

                # ---- normalize: h = num / (den + eps)
                den_t = wpool.tile([P, H], f32, tag="den")
                nc.scalar.activation(out=den_t[:], in_=acc_ps[:, HD:C],
                                     func=AF.Identity, bias=eps_sb[:, 0:1], scale=1.0)
                rec_t = wpool.tile([P, H], f32, tag="rec")
                nc.vector.reciprocal(out=rec_t[:], in_=den_t[:])
                h_t = wpool.tile([P, HD], f32, tag="h")
                nc.vector.tensor_tensor(
                    out=h_t[:].rearrange("p (h d) -> p h d", h=H),
                    in0=acc_ps[:, 0:HD].rearrange("p (h d) -> p h d", h=H),
                    in1=rec_t[:].unsqueeze(2).to_broadcast([P, H, D]),
                    op=mybir.AluOpType.mult)

                # ---- transpose h, add layer bias -> hT[f, n]
                hT_ps = ppool2.tile([P, P], f32, tag="hT")
                nc.tensor.transpose(out=hT_ps[:], in_=h_t[:], identity=ident_sb[:])
                hT_sb = wpool.tile([P, P], f32, tag="hTs")
                nc.scalar.activation(out=hT_sb[:], in_=hT_ps[:], func=AF.Identity,
                                     bias=bcol_sb[:, 0:1], scale=1.0)

                if phase == 2:
                    g1_ps = ppool1.tile([P, HD], f32, tag="g1")
                    nc.tensor.matmul(out=g1_ps[:], lhsT=hT_sb[:], rhs=W1_sb[:],
                                     start=True, stop=True)
                    asd_ps = ppool1.tile([P, 2 * H], f32, tag="asd1")
                    nc.tensor.matmul(out=asd_ps[:], lhsT=hT_sb[:], rhs=Was1_sb[:],
                                     start=True, stop=True)
                    g_st = wpool.tile([P, HD], f32, tag="g_st")
                    nc.vector.tensor_copy(out=g_st[:], in_=g1_ps[:])
                    asd_st = wpool.tile([P, 2 * H], f32, tag="asd_st")
                    nc.scalar.activation(out=asd_st[:], in_=asd_ps[:],
                                         func=AF.Identity, bias=0.0, scale=1.0)
                    nc.sync.dma_start(out=g_out[ts(b, P), :], in_=g_st[:])
                    nc.sync.dma_start(out=asd_out[ts(b, P), :], in_=asd_st[:])
                else:
                    ce_ps = ppool1.tile([1, P], f32, tag="cep")
                    nc.tensor.matmul(out=ce_ps[:], lhsT=dWy_sb[:], rhs=hT_sb[:],
                                     start=True, stop=True)
                    nc.scalar.activation(out=ce_st[0:1, ts(b, P)], in_=ce_ps[0:1, :],
                                         func=AF.Identity, bias=dby_sb[0:1, 0:1],
                                         scale=1.0)
                    c1_ps = ppool1.tile([D, P], f32, tag="c1p")
                    nc.tensor.matmul(out=c1_ps[:], lhsT=Wc1_sb[:], rhs=hT_sb[:],
                                     start=True, stop=True)
                    c1_sb = wpool.tile([D, P], f32, tag="c1")
                    nc.scalar.activation(out=c1_sb[:], in_=c1_ps[:],
                                         func=AF.Relu, bias=bc1_sb[:, 0:1], scale=1.0)
                    pr_ps = ppool1.tile([1, P], f32, tag="pr")
                    nc.tensor.matmul(out=pr_ps[:], lhsT=Wc2_sb[:], rhs=c1_sb[:],
                                     start=True, stop=True)
                    nc.scalar.activation(out=prop_st[0:1, ts(b, P)], in_=pr_ps[0:1, :],
                                         func=AF.Sigmoid, bias=bc2_sb[0:1, 0:1],
                                         scale=1.0)

            for b in range(NB):
                body(b)

            if phase == 3:
                nc.sync.dma_start(out=ce_out[0:1, :], in_=ce_st[:])
                nc.sync.dma_start(out=prop_out[0:1, :], in_=prop_st[:])
    nc.compile()
    return nc


# =====================================================================
# execution helpers
# =====================================================================

def _run(nc, in_maps, label):
    backend = os.environ.get("KERNEL_BACKEND", "hw")
    if backend == "sim":
        from concourse.bass_interp import CoreSim
        outs = []
        for m in in_maps:
            sim = CoreSim(nc, trace=False)
            for k, v in m.items():
                sim.tensor(k)[:] = v
            sim.simulate(check_with_hw=False)
            out_names = []
            for alloc in nc.m.functions[0].allocations:
                if isinstance(alloc, mybir.MemoryLocationSet) and alloc.kind == "ExternalOutput":
                    out_names.append(alloc.memorylocations[0].name)
            outs.append({k: np.array(sim.tensor(k)) for k in out_names})
        return outs
    from concourse.bass_utils import run_bass_kernel_spmd
    trace = os.environ.get("KERNEL_TRACE", "0") == "1"
    res = run_bass_kernel_spmd(nc, in_maps, core_ids=list(range(len(in_maps))),
                               trace=trace)
    if res.exec_time_ns is not None:
        LAST_EXEC_TIMES[label] = res.exec_time_ns
    LAST_TRACES[label] = res.instructions_and_trace
    return res.results


# =====================================================================
# host-side routing / layout
# =====================================================================

def _att_flat(att):
    """[H, D] attention vector -> [HD, H] block matrix."""
    out = np.zeros((HD, H), np.float32)
    for h in range(H):
        out[h * D:(h + 1) * D, h] = att[h]
    return out


def _route_edges(cfg, src, dst):
    """Sort edges by dst, bucket into (core, block, subtile k, lane p) slots.

    Returns K and per-core [npc, K] int32 src ids, [npc, K] f32 local dst
    (pad slots get 300.0 so the on-chip selection matrix zeroes them), and
    the global [ncore, nb*K*128] slot->src / slot->dst maps used to expand
    per-edge streams.
    """
    order = np.argsort(dst, kind="stable")
    src_s = src[order]
    dst_s = dst[order]
    nblk = cfg.ncores * cfg.nb
    blk = dst_s // P
    cnt = np.bincount(blk, minlength=nblk)
    K = max(1, int(math.ceil(cnt.max() / P)))
    start = np.zeros(nblk, np.int64)
    np.cumsum(cnt[:-1], out=start[1:])
    rank = np.arange(len(dst_s), dtype=np.int64) - start[blk]
    kk = rank // P
    pp = rank % P

    src_slot = np.zeros((nblk, K, P), np.int32)
    dst_slot = np.zeros((nblk, K, P), np.int32)
    dstl_slot = np.full((nblk, K, P), 300.0, np.float32)
    src_slot[blk, kk, pp] = src_s
    dst_slot[blk, kk, pp] = dst_s
    dstl_slot[blk, kk, pp] = (dst_s - blk * P).astype(np.float32)
    return K, src_slot, dst_slot, dstl_slot


def _per_core_idx(cfg, src_slot, dstl_slot):
    """[nblk, K, P] -> per-core [npc, K] (row = b*128+p, col = k)."""
    idx_c, dstl_c = [], []
    for c in range(cfg.ncores):
        sl = slice(c * cfg.nb, (c + 1) * cfg.nb)
        # [nb, K, P] -> [nb, P, K] -> [npc, K]
        idx_c.append(np.ascontiguousarray(
            src_slot[sl].transpose(0, 2, 1).reshape(cfg.npc, -1)))
        dstl_c.append(np.ascontiguousarray(
            dstl_slot[sl].transpose(0, 2, 1).reshape(cfg.npc, -1)))
    return idx_c, dstl_c


def _streams(cfg, asd_full, src_slot, dst_slot):
    """Per-edge (a_src[src] | a_dst[dst]) stream, [npc, K*16] per core."""
    outs = []
    for c in range(cfg.ncores):
        sl = slice(c * cfg.nb, (c + 1) * cfg.nb)
        s = src_slot[sl]            # [nb, K, P]
        d = dst_slot[sl]
        st = np.empty(s.shape + (2 * H,), np.float32)   # [nb, K, P, 16]
        st[..., 0:H] = asd_full[s, 0:H]
        st[..., H:2 * H] = asd_full[d, H:2 * H]
        # -> [nb, P, K, 16] -> [npc, K*16]
        outs.append(np.ascontiguousarray(
            st.transpose(0, 2, 1, 3).reshape(cfg.npc, -1)))
    return outs


# =====================================================================
# entry point
# =====================================================================

def kernel(x, edge_index, W_emb, b_emb, W0, as0, ad0, b0,
           W1, as1, ad1, b1, Wy1, by1, Wy0, by0, Wc1, bc1, Wc2, bc2,
           cfg=None):
    cfg = cfg or CFG
    np_f = lambda a: np.asarray(a, np.float32)
    x = np_f(x)
    n = x.shape[0]
    assert n == cfg.n

    # ---- self loops + routing
    loop = np.arange(n, dtype=np.int64)
    src = np.concatenate([np.asarray(edge_index[0], np.int64), loop]).astype(np.int32)
    dst = np.concatenate([np.asarray(edge_index[1], np.int64), loop]).astype(np.int32)
    K, src_slot, dst_slot, dstl_slot = _route_edges(cfg, src, dst)
    idx_c, dstl_c = _per_core_idx(cfg, src_slot, dstl_slot)

    # ---- constant folds
    W_emb, W0, W1 = np_f(W_emb), np_f(W0), np_f(W1)
    Was0 = np.concatenate([W0 @ _att_flat(np_f(as0)), W0 @ _att_flat(np_f(ad0))], 1)
    Was1 = np.concatenate([W1 @ _att_flat(np_f(as1)), W1 @ _att_flat(np_f(ad1))], 1)
    dWy = np_f(Wy1) - np_f(Wy0)                                   # [128, 1]
    dby = (np_f(by1) - np_f(by0)).reshape(1, 1)
    iota = np.tile(np.arange(P, dtype=np.float32), (P, 1))
    ident = np.eye(P, dtype=np.float32)

    # ---- phase 1
    x_pad = np.zeros((cfg.npad, F), np.float32)
    x_pad[:n] = x
    nc1 = _build_phase1(cfg)
    maps1 = []
    for c in range(cfg.ncores):
        xT = np.ascontiguousarray(x_pad[c * cfg.npc:(c + 1) * cfg.npc].T)
        maps1.append(dict(xT=xT, Wemb=W_emb, bemb=np_f(b_emb).reshape(D, 1),
                          W0=W0, Was0=Was0))
    res1 = _run(nc1, maps1, "phase1")
    g0 = np.concatenate([r["g_out"] for r in res1], 0)         # [npad, 128]
    asd0 = np.concatenate([r["asd_out"] for r in res1], 0)     # [npad, 16]
    DEBUG_STASH.update(g0=g0, asd0=asd0)

    # ---- phase 2 (GAT layer 0)
    st0 = _streams(cfg, asd0, src_slot, dst_slot)
    nc2 = _build_gat(cfg, K, 2)
    maps2 = []
    for c in range(cfg.ncores):
        maps2.append(dict(gtable=g0, idx=idx_c[c], dstl=dstl_c[c], stream=st0[c],
                          iota=iota, ident=ident, bcol=np_f(b0).reshape(HD, 1),
                          W1=W1, Was1=Was1))
    res2 = _run(nc2, maps2, "phase2")
    g1 = np.concatenate([r["g_out"] for r in res2], 0)
    asd1 = np.concatenate([r["asd_out"] for r in res2], 0)
    DEBUG_STASH.update(g1=g1, asd1=asd1)

    # ---- phase 3 (GAT layer 1 + heads)
    st1 = _streams(cfg, asd1, src_slot, dst_slot)
    nc3 = _build_gat(cfg, K, 3)
    maps3 = []
    for c in range(cfg.ncores):
        maps3.append(dict(gtable=g1, idx=idx_c[c], dstl=dstl_c[c], stream=st1[c],
                          iota=iota, ident=ident, bcol=np_f(b1).reshape(HD, 1),
                          dWy=dWy, Wc1=np_f(Wc1), dby=dby,
                          bc1=np_f(bc1).reshape(D, 1),
                          bc2=np_f(bc2).reshape(1, 1), Wc2=np_f(Wc2)))
    res3 = _run(nc3, maps3, "phase3")
    ce = np.concatenate([r["ce_out"][0] for r in res3])[:n].reshape(n, 1)
    prop = np.concatenate([r["prop_out"][0] for r in res3])[:n].reshape(n, 1)
    return ce.astype(np.float32), prop.astype(np.float32)

